# revision 9
# baseline (speedup 1.0000x reference)
"""Weighted-BCE + masked-MSE loss on 8 Trainium2 cores (pure data parallel).

Inputs cast to fp16 host-side during sharding (t is {0,1} exact; p is
clamped to [2^-10, 1-2^-11] and pre-biased to p' = p-1 so the device
can form u = t + p' with a single plain tensor_tensor add):

  u = t + p'             -> |u| = p if t==1 else 1-p
  lnc = ln|u|            -> s_lnc = sum(lnc); s_tl = sum(t*lnc) recovers
                            both weighted-BCE log sums
  e = t*dd; g = e - dd   -> g = (t-1)*dd, g^2 = (1-t)*dd^2 (masked MSE)
  count = sum(t), s_tl   -> PE matmuls against a ones vector

Engine mix per tile (only op types validated fast on this stack):
  Pool: dd = ro - rt
  DVE : u, e, g, tl = t*lnc      [4 plain TT ops, fp16]
  ACT : c = |u|; Ln(c)+accum; Square(g)+accum
  PE  : ones^T @ t and ones^T @ tl chunks
"""

import os
import sys

for _p in ("/opt/trn_rl_repo", "/root/.axon_site/_ro/trn_rl_repo"):
    if os.path.isdir(_p) and _p not in sys.path:
        sys.path.insert(0, _p)

import numpy as np

import concourse.bacc as bacc
import concourse.mybir as mybir
from concourse import tile
from concourse.bass_utils import run_bass_kernel_spmd

N = 16777216
NCORES = 8
NSHARD = N // NCORES  # 2097152
P = 128
F = 4096
NT = NSHARD // (P * F)  # 4

_F32 = mybir.dt.float32
_F16 = mybir.dt.float16

LAST_RESULTS = None  # test harness peeks at exec_time_ns / trace path


def _build_nc():
    AF = mybir.ActivationFunctionType
    OP = mybir.AluOpType
    AX = mybir.AxisListType

    nc = bacc.Bacc(
        "TRN2", target_bir_lowering=False, debug=False, num_devices=NCORES
    )
    p_d = nc.dram_tensor("p", [NT, P, F], _F16, kind="ExternalInput")
    t_d = nc.dram_tensor("t", [NT, P, F], _F16, kind="ExternalInput")
    ro_d = nc.dram_tensor("ro", [NT, P, F], _F16, kind="ExternalInput")
    rt_d = nc.dram_tensor("rt", [NT, P, F], _F16, kind="ExternalInput")
    out_d = nc.dram_tensor("out", [1, 4], _F32, kind="ExternalOutput")

    with tile.TileContext(nc) as tc:
        with (
            tc.tile_pool(name="io", bufs=2) as io,
            tc.tile_pool(name="work", bufs=2) as work,
            tc.tile_pool(name="junkp", bufs=1) as junkp,
            tc.tile_pool(name="stats", bufs=1) as stats,
            tc.tile_pool(name="psum", bufs=1, space="PSUM") as psum,
        ):
            # acc columns: [0:NT]=sum(lnc), [NT:2NT]=sum(g^2)
            acc = stats.tile([P, 2 * NT], _F32)

            ones16 = stats.tile([P, 1], _F16)
            nc.vector.memset(ones16[:], 1.0)
            psum_cnt = psum.tile([1, 512], _F32)
            psum_tl = psum.tile([1, 512], _F32)
            NCHUNK = F // 512

            for i in range(NT):
                tp = io.tile([P, F], _F16, tag="p")
                tt = io.tile([P, F], _F16, tag="t")
                tro = io.tile([P, F], _F16, tag="ro")
                trt = io.tile([P, F], _F16, tag="rt")
                nc.sync.dma_start(tp[:], p_d[i, :, :])
                nc.sync.dma_start(tt[:], t_d[i, :, :])
                nc.sync.dma_start(tro[:], ro_d[i, :, :])
                nc.sync.dma_start(trt[:], rt_d[i, :, :])

                # DVE: u = t' + p  (t' = t-1 staged on host; u = p or p-1)
                u = work.tile([P, F], _F16, tag="u")
                nc.vector.tensor_tensor(u[:], tt[:], tp[:], OP.add)

                # DVE: dd = ro - rt
                dd = work.tile([P, F], _F16, tag="dd")
                nc.vector.tensor_tensor(dd[:], tro[:], trt[:], OP.subtract)
                # DVE: v = u*u (= c^2, always normal fp16 given the p clamps)
                v = work.tile([P, F], _F16, tag="c")
                nc.vector.tensor_tensor(v[:], u[:], u[:], OP.mult)
                # ACT: lnc = Ln(u^2) = 2*ln|u| with free column-sum
                lnc = work.tile([P, F], _F16, tag="lnc")
                nc.scalar.activation(
                    lnc[:], v[:], AF.Ln, accum_out=acc[:, i : i + 1]
                )
                # DVE: tl = t * lnc (summed on PE below)
                tl = work.tile([P, F], _F16, tag="tl")
                nc.vector.tensor_tensor(tl[:], tt[:], lnc[:], OP.mult)

                # DVE: g = t'*dd = (t-1)*dd directly
                g = work.tile([P, F], _F16, tag="g")
                nc.vector.tensor_tensor(g[:], tt[:], dd[:], OP.mult)
                junk2 = junkp.tile([P, 1], _F32, tag="junk2")
                nc.scalar.activation(
                    junk2[:].broadcast_to([P, F]), g[:], AF.Square,
                    accum_out=acc[:, NT + i : NT + i + 1],
                )

                # PE: column-sums of t (count) and tl (sum t*lnc)
                for cch in range(NCHUNK):
                    sl = slice(cch * 512, (cch + 1) * 512)
                    nc.tensor.matmul(
                        psum_cnt[0:1, :], ones16[:, 0:1], tt[:, sl],
                        start=(i == 0 and cch == 0),
                        stop=(i == NT - 1 and cch == NCHUNK - 1),
                    )
                    nc.tensor.matmul(
                        psum_tl[0:1, :], ones16[:, 0:1], tl[:, sl],
                        start=(i == 0 and cch == 0),
                        stop=(i == NT - 1 and cch == NCHUNK - 1),
                    )

            # Epilogue: fold per-tile partials into out[1,4]
            red = stats.tile([P, 2], _F32)
            for j in range(2):
                nc.vector.tensor_reduce(
                    red[:, j : j + 1], acc[:, j * NT : (j + 1) * NT], AX.X, OP.add
                )
            ones32 = stats.tile([P, 1], _F32)
            nc.vector.memset(ones32[:], 1.0)
            psum_out = psum.tile([1, 4], _F32)
            nc.tensor.matmul(
                psum_out[0:1, 0:2], ones32[:, 0:1], red[:, 0:2],
                start=True, stop=True,
            )
            outsb = stats.tile([1, 4], _F32)
            nc.vector.tensor_copy(outsb[0:1, 0:2], psum_out[0:1, 0:2])
            nc.vector.tensor_reduce(
                outsb[0:1, 2:3], psum_cnt[0:1, :], AX.X, OP.add
            )
            nc.vector.tensor_reduce(
                outsb[0:1, 3:4], psum_tl[0:1, :], AX.X, OP.add
            )
            nc.sync.dma_start(out_d[:], outsb[0:1, 0:4])

    nc.compile()
    return nc


def kernel(class_output, reg_output, class_target, reg_target, class_weights):
    global LAST_RESULTS
    nc = _build_nc()

    # fp16 staging: clamp p into [2^-10, 1-2^-11], then pre-bias to p-1
    f16 = np.float16
    pc = np.clip(
        np.asarray(class_output).astype(f16), f16(2.0**-7), f16(1.0 - 2.0**-7)
    )
    t16 = np.asarray(class_target).astype(f16) - f16(1.0)  # t' = t-1 in {-1,0}
    ro16 = np.asarray(reg_output).astype(f16)
    rt16 = np.asarray(reg_target).astype(f16)

    def shards(a):
        a = np.ascontiguousarray(a)
        return [
            a[c * NSHARD : (c + 1) * NSHARD].reshape(NT, P, F) for c in range(NCORES)
        ]

    ps, ts, ros, rts = shards(pc), shards(t16), shards(ro16), shards(rt16)
    in_maps = [
        {"p": ps[c], "t": ts[c], "ro": ros[c], "rt": rts[c]} for c in range(NCORES)
    ]

    res = run_bass_kernel_spmd(nc, in_maps, core_ids=list(range(NCORES)))
    LAST_RESULTS = res

    parts = np.stack([np.asarray(res.results[c]["out"][0]) for c in range(NCORES)])
    tot = parts.sum(axis=0, dtype=np.float64)
    s_lv, s_msq, s_tp, s_tlp = tot
    s_lnc = 0.5 * s_lv              # Ln ran on u^2
    s_tl = 0.5 * (s_tlp + s_lv)     # sum(t*lv) = sum(t'*lv) + sum(lv)
    s_t = s_tp + N                  # sum(t) = sum(t-1) + N

    w0 = float(np.asarray(class_weights)[0, 0])
    w1 = float(np.asarray(class_weights)[0, 1])
    class_loss = -(w1 * s_tl + w0 * (s_lnc - s_tl)) / N
    cnt = N - s_t
    reg_loss = (s_msq / cnt) if cnt > 0 else 0.0
    return np.float32(0.5 * class_loss + 0.5 * reg_loss)
